# revision 17
# baseline (speedup 1.0000x reference)
"""Cross-attention Trainium2 kernel (8 NeuronCores, SPMD).

Problem: B=4, S=3072, SKV=1036, D_EMBED=1024, D_CROSS=768, H=8, d_head=128.

Sharding: core c -> (batch b = c//2, head-group hg = c%2 of 4 heads).
Each core computes the full S=3072 query rows for its 4 heads and the
out-projection PARTIAL out_hg = attn_hg @ Wo.T[hg_slice]; the host gather
sums the two partials per batch (+ const row for the bv/bo biases).

All inputs are pre-cast to bf16 on the host (so every DMA is non-casting
and rides the idle sync engine DGE). The whole kernel is one interleaved
stream: K-proj, V-proj, Q-proj(0), then an sc-outer attention loop that
injects Q-proj(sc+1) and out-proj(sc-1) matmuls between attention
iterations so the PE never drains. Per attention iteration (h, sc):
  scoresT[t,s] = kT_h.T @ qT_h_chunk       (9 mm -> psum ring)
  p = exp(scoresT/sqrt(128))               (ACT -> bf16, 5 instrs)
  PV: pso = sum_t v_h.T @ p                (9 mm)
  Z tree: p01,p23,p0123 on Pool; p45,p67,p4567,pall on DVE (bf16)
  one stage later: zf = ones@pall + ones12@p_rem (2 mm), zrb=recip(zf),
  attnT_h[:,sc] = pso * zrb                (DVE fused norm-evac, bf16)
Out-proj psum shares one 2-bank ring with Q-proj psum; ACT evacuates Q/K
(with bias), DVE evacuates V/attn/out tiles.
"""

import sys

sys.path.insert(0, "/opt/trn_rl_repo")

import math

import numpy as np
import ml_dtypes

import concourse.bass as bass
import concourse.mybir as mybir
import concourse.tile as tile
from concourse import bacc
from concourse.bass import ts, ds
from concourse.bass_utils import run_bass_kernel_spmd

N_CORES = 8
B, S, SKV = 4, 3072, 1036
DE, DC, H, DH = 1024, 768, 8, 128
HG = 512               # head-group width (4 heads x 128)
NEh = HG // 128        # 4 d-tiles per head group (= local heads)
NK = DE // 128         # 8 x-contraction tiles
NCC = DC // 128        # 6 y-contraction tiles
NT_FULL = SKV // 128   # 8 full t-tiles
T_REM = SKV - NT_FULL * 128  # 12
NT = NT_FULL + 1       # 9 t-tiles
NSC = S // 512         # 6 s-chunks
NS = S // 128          # 24 s-tiles
INV_SQRT_DH = 1.0 / math.sqrt(DH)

F32 = mybir.dt.float32
BF16 = mybir.dt.bfloat16
ADD = mybir.AluOpType.add
MULT = mybir.AluOpType.mult
IDENT = mybir.ActivationFunctionType.Identity
EXP = mybir.ActivationFunctionType.Exp


def _tw(ti):
    return 128 if ti < NT_FULL else T_REM


def build_bass():
    nc = bacc.Bacc("TRN2", target_bir_lowering=False, debug=False)

    xT_d = nc.dram_tensor("xT", [DE, S], BF16, kind="ExternalInput").ap()
    yT_d = nc.dram_tensor("yT", [DC, SKV], BF16, kind="ExternalInput").ap()
    wqT_d = nc.dram_tensor("WqT", [DE, HG], BF16, kind="ExternalInput").ap()
    wkT_d = nc.dram_tensor("WkT", [DC, HG], BF16, kind="ExternalInput").ap()
    wvT_d = nc.dram_tensor("WvT", [DC, HG], BF16, kind="ExternalInput").ap()
    woT_d = nc.dram_tensor("WoT", [HG, DE], BF16, kind="ExternalInput").ap()
    bq_d = nc.dram_tensor("bq", [HG], F32, kind="ExternalInput").ap()
    bk_d = nc.dram_tensor("bk", [HG], F32, kind="ExternalInput").ap()
    out_d = nc.dram_tensor("out", [S, DE], F32, kind="ExternalOutput").ap()

    with tile.TileContext(nc) as tc:
        with (
            tc.tile_pool(name="misc", bufs=1) as misc,
            tc.tile_pool(name="keep", bufs=1) as keep,
            tc.tile_pool(name="xch_in", bufs=2) as xch_in,
            tc.tile_pool(name="o_out", bufs=3) as o_out,
            tc.tile_pool(name="awork", bufs=2) as awork,
            tc.tile_pool(name="ps_qo", bufs=2, space="PSUM") as ps_qo,
        ):
            ones_sb = misc.tile([128, 128], BF16)
            nc.any.memset(ones_sb, 1.0)
            bq_sb = misc.tile([128, NEh], F32)
            nc.sync.dma_start(bq_sb, bq_d.rearrange("(j p) -> p j", p=128))
            bk_sb = misc.tile([128, NEh], F32)
            nc.sync.dma_start(bk_sb, bk_d.rearrange("(j p) -> p j", p=128))

            qT = keep.tile([128, NEh, S], BF16)
            kT = keep.tile([128, NEh, SKV], BF16)
            v_sb = keep.tile([128, NT, HG], BF16)
            attnT = keep.tile([128, NEh, S], BF16)
            woT = keep.tile([128, NEh, DE], BF16)
            wqT = keep.tile([128, NK, HG], BF16)
            # wq/wo ride the scalar DGE queue so the sync/gpsimd queues can
            # deliver y/wk (the first tensors PE needs) with minimum latency;
            # one strided DMA per tensor keeps descriptor-gen off the
            # startup critical path
            nc.scalar.dma_start(wqT, wqT_d.rearrange("(k p) h -> p k h", p=128))
            nc.scalar.dma_start(woT, woT_d.rearrange("(k p) e -> p k e", p=128))

            xT_r = xT_d.rearrange("(k p) s -> p k s", p=128)

            def dma_xch(sc):
                xch = xch_in.tile([128, NK, 512], BF16, tag="xch")
                nc.sync.dma_start(xch, xT_r[:, :, ts(sc, 512)])
                return xch

            def q_proj(sc, xch):
                for di in range(NEh):
                    ps = ps_qo.tile([128, 512], F32, tag="qo")
                    for ki in range(NK):
                        nc.tensor.matmul(
                            ps, wqT[:, ki, ts(di, 128)], xch[:, ki],
                            start=(ki == 0), stop=(ki == NK - 1),
                        )
                    nc.scalar.activation(
                        qT[:, di, ts(sc, 512)], ps, IDENT,
                        bias=bq_sb[:, ds(di, 1)],
                    )

            # ---- setup: K, V (own psum pool, released after), Q(0) ----
            wkv = tc.alloc_tile_pool(name="wkv", bufs=1)
            yT_sb = wkv.tile([128, NCC, SKV], BF16)
            wkT = wkv.tile([128, NCC, HG], BF16)
            wvT = wkv.tile([128, NCC, HG], BF16)
            # first halves of y land first so K-proj's t<512 chunk can start
            # while the rest of the inputs stream in
            yT_r = yT_d.rearrange("(c p) t -> p c t", p=128)
            nc.sync.dma_start(yT_sb[:, :, 0:512], yT_r[:, :, 0:512])
            nc.gpsimd.dma_start(wkT, wkT_d.rearrange("(c p) h -> p c h", p=128))
            nc.sync.dma_start(yT_sb[:, :, 512:SKV], yT_r[:, :, 512:SKV])
            nc.gpsimd.dma_start(wvT, wvT_d.rearrange("(c p) h -> p c h", p=128))
            xch0 = dma_xch(0)
            ps_kv = tc.alloc_tile_pool(name="ps_kv", bufs=6, space="PSUM")
            for tci in range(3):
                t0 = tci * 512
                tw = min(512, SKV - t0)
                for di in range(NEh):
                    ps = ps_kv.tile([128, 512], F32, tag="kv")
                    for ci in range(NCC):
                        nc.tensor.matmul(
                            ps[:, :tw], wkT[:, ci, ts(di, 128)],
                            yT_sb[:, ci, ds(t0, tw)],
                            start=(ci == 0), stop=(ci == NCC - 1),
                        )
                    nc.scalar.activation(
                        kT[:, di, ds(t0, tw)], ps[:, :tw], IDENT,
                        bias=bk_sb[:, ds(di, 1)],
                    )
            for ti in range(NT):
                tw = _tw(ti)
                ps = ps_kv.tile([128, 512], F32, tag="kv")
                for ci in range(NCC):
                    nc.tensor.matmul(
                        ps[:tw], yT_sb[:, ci, ds(ti * 128, tw)], wvT[:, ci],
                        start=(ci == 0), stop=(ci == NCC - 1),
                    )
                nc.vector.tensor_copy(v_sb[:tw, ti], ps[:tw])
            q_proj(0, xch0)
            ps_kv.release()
            wkv.release()

            ps_s = tc.alloc_tile_pool(name="ps_s", bufs=2, space="PSUM")
            ps_o = tc.alloc_tile_pool(name="ps_o", bufs=2, space="PSUM")

            # ---- attention + interleaved Q(sc+1) / O(sc-1) ----
            def stage_scores(h, sc):
                expT = awork.tile([128, NT, 512], BF16, tag="expT", bufs=3)
                for pi in range(5):
                    ps = ps_s.tile([128, 2, 512], F32, tag="pss")
                    nj = 2 if pi < 4 else 1
                    for j in range(nj):
                        ti = pi * 2 + j
                        tw = _tw(ti)
                        nc.tensor.matmul(
                            ps[:tw, j], kT[:, h, ds(ti * 128, tw)],
                            qT[:, h, ts(sc, 512)],
                            start=True, stop=True,
                        )
                    if nj == 2:
                        nc.scalar.activation(
                            expT[:, ts(pi, 2)], ps, EXP, scale=INV_SQRT_DH
                        )
                    else:
                        nc.scalar.activation(
                            expT[:T_REM, NT_FULL], ps[:T_REM, 0], EXP,
                            scale=INV_SQRT_DH,
                        )
                return (expT,)

            def stage_pv(h, sc, expT):
                pso = ps_o.tile([128, 512], F32, tag="pso")
                for ti in range(NT):
                    tw = _tw(ti)
                    nc.tensor.matmul(
                        pso, v_sb[:tw, ti, ds(h * 128, 128)], expT[:tw, ti],
                        start=(ti == 0), stop=(ti == NT - 1),
                    )
                p01 = awork.tile([128, 512], BF16, tag="p01")
                p23 = awork.tile([128, 512], BF16, tag="p23")
                p0123 = awork.tile([128, 512], BF16, tag="p0123")
                nc.gpsimd.tensor_tensor(p01, expT[:, 0], expT[:, 1], op=ADD)
                nc.gpsimd.tensor_tensor(p23, expT[:, 2], expT[:, 3], op=ADD)
                nc.gpsimd.tensor_tensor(p0123, p01, p23, op=ADD)
                p45 = awork.tile([128, 512], BF16, tag="p45")
                p67 = awork.tile([128, 512], BF16, tag="p67")
                pall = awork.tile([128, 512], BF16, tag="pall")
                nc.vector.tensor_tensor(p45, expT[:, 4], expT[:, 5], op=ADD)
                nc.vector.tensor_tensor(p67, expT[:, 6], expT[:, 7], op=ADD)
                nc.vector.tensor_tensor(pall, p45, p67, op=ADD)
                nc.vector.tensor_tensor(pall, pall, p0123, op=ADD)
                # fold the 12-row t-remainder into pall's first partitions so
                # Z needs a single ones-matmul (column sums span all 128)
                nc.vector.tensor_tensor(
                    pall[:T_REM], pall[:T_REM], expT[:T_REM, NT_FULL], op=ADD
                )
                return (pso, pall, expT)

            def stage_norm(h, sc, pso, pall, expT):
                zf = ps_s.tile([128, 2, 512], F32, tag="pss")
                nc.tensor.matmul(zf[:, 0], ones_sb, pall, start=True, stop=True)
                zrb = awork.tile([128, 512], F32, tag="zrb")
                nc.vector.reciprocal_approx_fast(zrb, zf[:, 0])
                nc.vector.tensor_tensor(
                    attnT[:, h, ts(sc, 512)], pso, zrb, op=MULT
                )

            def o_proj(sc):
                for sj in range(4):
                    si = sc * 4 + sj
                    out_sb = o_out.tile([128, DE], F32, tag="osb")
                    for ec in range(2):
                        ps = ps_qo.tile([128, 512], F32, tag="qo")
                        for dl in range(NEh):
                            nc.tensor.matmul(
                                ps, attnT[:, dl, ts(si, 128)],
                                woT[:, dl, ts(ec, 512)],
                                start=(dl == 0), stop=(dl == NEh - 1),
                            )
                        nc.vector.tensor_copy(out_sb[:, ts(ec, 512)], ps)
                    nc.sync.dma_start(out_d[ts(si, 128)], out_sb)

            xch_next = dma_xch(1)
            pipeA = None
            pipeB = None
            for sc in range(NSC):
                for h in range(NEh):
                    if h == 0 and sc < NSC - 2:
                        xch_nn = dma_xch(sc + 2)
                    if h == 2 and sc < NSC - 1:
                        q_proj(sc + 1, xch_next)
                        if sc < NSC - 2:
                            xch_next = xch_nn
                    curA = (h, sc, *stage_scores(h, sc))
                    if pipeB is not None:
                        stage_norm(*pipeB)
                        pipeB = None
                    if pipeA is not None:
                        pipeB = (pipeA[0], pipeA[1], *stage_pv(*pipeA))
                    pipeA = curA
                    if h == 3 and sc >= 1:
                        o_proj(sc - 1)
            pipeB2 = (pipeA[0], pipeA[1], *stage_pv(*pipeA))
            stage_norm(*pipeB)
            stage_norm(*pipeB2)
            o_proj(NSC - 1)
            ps_o.release()
            ps_s.release()

    nc.compile()
    return nc


_NC_CACHE = None


def _get_nc():
    global _NC_CACHE
    if _NC_CACHE is None:
        _NC_CACHE = build_bass()
    return _NC_CACHE


def make_in_maps(inputs):
    bf = ml_dtypes.bfloat16
    x = np.asarray(inputs["x"], np.float32)
    y = np.asarray(inputs["y"], np.float32)
    Wq = np.asarray(inputs["Wq"], np.float32)
    Wk = np.asarray(inputs["Wk"], np.float32)
    Wv = np.asarray(inputs["Wv"], np.float32)
    Wo = np.asarray(inputs["Wo"], np.float32)
    bq = np.asarray(inputs["bq"], np.float32)
    bk = np.asarray(inputs["bk"], np.float32)

    xTs = [np.ascontiguousarray(x[b].T).astype(bf) for b in range(B)]
    yTs = [np.ascontiguousarray(y[b].T).astype(bf) for b in range(B)]
    per_hg = []
    for hg in range(2):
        hs = slice(hg * HG, (hg + 1) * HG)
        per_hg.append(
            {
                "WqT": np.ascontiguousarray(Wq[hs, :].T).astype(bf),
                "WkT": np.ascontiguousarray(Wk[hs, :].T).astype(bf),
                "WvT": np.ascontiguousarray(Wv[hs, :].T).astype(bf),
                "WoT": np.ascontiguousarray(Wo[:, hs].T).astype(bf),
                "bq": np.ascontiguousarray(bq[hs]),
                "bk": np.ascontiguousarray(bk[hs]),
            }
        )
    in_maps = []
    for c in range(N_CORES):
        b, hg = c // 2, c % 2
        in_maps.append({"xT": xTs[b], "yT": yTs[b], **per_hg[hg]})
    return in_maps


def gather(results, inputs):
    Wo = np.asarray(inputs["Wo"], np.float32)
    bv = np.asarray(inputs["bv"], np.float32)
    bo = np.asarray(inputs["bo"], np.float32)
    const = (Wo @ bv + bo).astype(np.float32)
    out = np.empty((B, S, DE), np.float32)
    for b in range(B):
        out[b] = results[2 * b]["out"] + results[2 * b + 1]["out"]
        out[b] += const
    return out


def kernel(**inputs) -> np.ndarray:
    nc = _get_nc()
    in_maps = make_in_maps(inputs)
    res = run_bass_kernel_spmd(nc, in_maps, core_ids=list(range(N_CORES)))
    return gather(res.results, inputs)
